# revision 9
# baseline (speedup 1.0000x reference)
"""Two-layer GCN (PyG GCNConv semantics) on 8 Trainium2 NeuronCores.

Strategy (dst-sharded graph parallelism):
  - Nodes are sharded by destination across the 8 cores (6250 dst nodes each).
  - Per layer, every core holds the full transformed-feature table in HBM as
    split-bf16 rows (hi + lo halves -> f32-grade precision), padded to 512B so
    a single dma_gather fetches one edge's message row per descriptor.
  - deg^-1/2 normalization is folded: source-side scale is pre-applied to the
    table rows, destination-side scale is applied per-partition after the
    segment sum.
  - Segment-sum (scatter-add by dst) runs on the TensorEngine: edges are
    host-sorted by dst and chopped into 128-edge groups; each group multiplies
    a one-hot "staircase" matrix (built on-device from a seg vector via
    tensor_tensor(is_equal)) against the gathered rows, accumulating into a
    PSUM tile per 128-dst-node block.
  - int16 gather indices cap the table at 32768 rows, so edges are split into
    A (src < 32768) and B (src >= 32768) streams with two table base offsets.
  - Layer 2 needs the full x1 table on every core; the x1' shards produced by
    NEFF-1 are routed through the host (pure concatenation, no host math) into
    NEFF-2's table input.
  - All cores run one SPMD program: per-tile group counts are maxed over
    cores (~8% padding) so the instruction stream is identical; pad slots
    gather row 0 with seg=-1 (matches no one-hot column => contributes 0).
"""

import hashlib
import math
from contextlib import ExitStack

import numpy as np
import ml_dtypes

import concourse.bacc as bacc
import concourse.tile as tile
from concourse import mybir
import concourse.bass_utils as bass_utils

BF16 = ml_dtypes.bfloat16

# ---------------- problem constants (hardcoded per contract) ----------------
N = 50000
F_IN = 96
HID = 96
F_OUT = 32
NCORES = 8
SPLIT = 32768          # int16 gather index limit
ROW = 256              # table row width in bf16 elems (512 B): hi[0:96] lo[96:192]
JA_CAP = 36            # max A-groups per gather call
JB_CAP = 24            # max B-groups per gather call


# ---------------- host-side metadata ----------------
class Meta:
    pass


def _pack_idx(flat):
    """[n] -> [128, n/16] int16; idx i at [i%16, i//16], replicated x8."""
    n = flat.shape[0]
    assert n % 128 == 0
    block = flat.reshape(n // 16, 16).T.astype(np.int16)
    return np.tile(block, (8, 1))


def build_meta(edge_index, n=N, ncores=NCORES, split=SPLIT,
               ja_cap=JA_CAP, jb_cap=JB_CAP):
    m = Meta()
    m.split = split
    src = np.asarray(edge_index[0]).astype(np.int64)
    dst = np.asarray(edge_index[1]).astype(np.int64)
    loops = np.arange(n, dtype=np.int64)
    src = np.concatenate([src, loops])
    dst = np.concatenate([dst, loops])

    deg = np.bincount(dst, minlength=n).astype(np.float32)
    dinv = (1.0 / np.sqrt(deg)).astype(np.float32)
    m.dinv = dinv

    shard = n // ncores
    ntiles = math.ceil(shard / 128)
    m.n, m.ncores, m.shard, m.ntiles = n, ncores, shard, ntiles

    order = np.argsort(dst, kind="stable")
    src_s = src[order]
    dst_s = dst[order]

    # per (core, tile): A/B src+seg lists (kept in dst order)
    lists = {}
    cntA = np.zeros((ncores, ntiles), np.int64)
    cntB = np.zeros((ncores, ntiles), np.int64)
    for c in range(ncores):
        for t in range(ntiles):
            start = c * shard + t * 128
            rows = min(128, shard - t * 128)
            lo = np.searchsorted(dst_s, start, "left")
            hi = np.searchsorted(dst_s, start + rows, "left")
            s_ct = src_s[lo:hi]
            g_ct = (dst_s[lo:hi] - start).astype(np.int64)
            ma = s_ct < split
            lists[(c, t, "A")] = (s_ct[ma], g_ct[ma])
            lists[(c, t, "B")] = (s_ct[~ma] - split, g_ct[~ma])
            cntA[c, t] = int(ma.sum())
            cntB[c, t] = int((~ma).sum())

    JA = np.ceil(cntA.max(axis=0) / 128).astype(np.int64)
    JB = np.ceil(cntB.max(axis=0) / 128).astype(np.int64)
    m.JA, m.JB = JA, JB

    # ranges: greedy pack tiles while within both caps
    ranges = []  # (t0, t1)
    t0 = 0
    while t0 < ntiles:
        t1 = t0 + 1
        sa, sb = JA[t0], JB[t0]
        while t1 < ntiles and sa + JA[t1] <= ja_cap and sb + JB[t1] <= jb_cap:
            sa += JA[t1]
            sb += JB[t1]
            t1 += 1
        ranges.append((t0, t1))
        t0 = t1
    m.ranges = ranges

    # global j-col offsets per tile
    offA = np.concatenate([[0], np.cumsum(JA)])
    offB = np.concatenate([[0], np.cumsum(JB)])
    m.offA, m.offB = offA, offB
    m.sumJA, m.sumJB = int(offA[-1]), int(offB[-1])

    # per-core padded flat arrays + packed tensors
    m.per_core = []
    for c in range(ncores):
        fa_idx = np.zeros(m.sumJA * 128, np.int64)
        fa_seg = np.full(m.sumJA * 128, -1.0, np.float32)
        fb_idx = np.zeros(m.sumJB * 128, np.int64)
        fb_seg = np.full(m.sumJB * 128, -1.0, np.float32)
        for t in range(ntiles):
            sa, ga = lists[(c, t, "A")]
            a0 = offA[t] * 128
            fa_idx[a0:a0 + len(sa)] = sa
            fa_seg[a0:a0 + len(sa)] = ga
            sb, gb = lists[(c, t, "B")]
            b0 = offB[t] * 128
            fb_idx[b0:b0 + len(sb)] = sb
            fb_seg[b0:b0 + len(sb)] = gb
        # idx tensors are packed per gather call
        ia_blocks, ib_blocks = [], []
        for (t0, t1) in ranges:
            ia_blocks.append(_pack_idx(fa_idx[offA[t0] * 128: offA[t1] * 128]))
            ib_blocks.append(_pack_idx(fb_idx[offB[t0] * 128: offB[t1] * 128]))
        idxA = np.concatenate(ia_blocks, axis=1) if m.sumJA else np.zeros((128, 0), np.int16)
        idxB = np.concatenate(ib_blocks, axis=1) if m.sumJB else np.zeros((128, 0), np.int16)
        segA = fa_seg.reshape(m.sumJA, 128).T.astype(BF16)
        segB = fb_seg.reshape(m.sumJB, 128).T.astype(BF16)
        # dst-side dinv per tile column
        dv = np.ones((128, ntiles), np.float32)
        for t in range(ntiles):
            rows = min(128, shard - t * 128)
            dv[:rows, t] = dinv[c * shard + t * 128: c * shard + t * 128 + rows]
        m.per_core.append(dict(idxA=idxA, idxB=idxB, segA=segA, segB=segB, dinv_sb=dv))

    m.iota = np.tile(np.arange(128, dtype=np.float32), (128, 1)).astype(BF16)
    m.ident = np.eye(128, dtype=np.float32)
    return m


# ---------------- shared aggregation emitter ----------------
def _emit_aggregation(nc, ctx, tc, meta, table_ap, consts, post_tile_fn):
    """Emit gather + staircase + PSUM segment-sum; call post_tile_fn(t, psum_tile,
    rows) after each dst-tile's accumulation completes."""
    split = meta.split
    idxA_t, idxB_t, segA_t, segB_t, iota_t = (
        consts["idxA"], consts["idxB"], consts["segA"], consts["segB"], consts["iota"])

    gpa = ctx.enter_context(tc.tile_pool(name="gA", bufs=2))
    gpb = ctx.enter_context(tc.tile_pool(name="gB", bufs=2))
    spa = ctx.enter_context(tc.tile_pool(name="sA", bufs=2))
    spb = ctx.enter_context(tc.tile_pool(name="sB", bufs=2))
    psum = ctx.enter_context(tc.tile_pool(name="agg", bufs=4, space="PSUM"))

    tabA = table_ap[0:split, :]
    tabB = table_ap[split:meta.n, :]

    for (t0, t1) in meta.ranges:
        jA0, jA1 = int(meta.offA[t0]), int(meta.offA[t1])
        jB0, jB1 = int(meta.offB[t0]), int(meta.offB[t1])
        JAr, JBr = jA1 - jA0, jB1 - jB0

        gA = sA = gB = sB = None
        if JAr:
            gA = gpa.tile([128, JAr, ROW], mybir.dt.bfloat16, tag="gA")
            nc.gpsimd.dma_gather(
                gA[:], tabA, idxA_t[:, jA0 * 8: jA1 * 8],
                num_idxs=JAr * 128, num_idxs_reg=JAr * 128, elem_size=ROW,
                single_packet=False)
            sA = spa.tile([128, JAr, 128], mybir.dt.bfloat16, tag="sA")
            nc.vector.tensor_tensor(
                sA[:],
                segA_t[:, jA0:jA1].unsqueeze(2).broadcast_to((128, JAr, 128)),
                iota_t[:].unsqueeze(1).broadcast_to((128, JAr, 128)),
                mybir.AluOpType.is_equal)
        if JBr:
            gB = gpb.tile([128, JBr, ROW], mybir.dt.bfloat16, tag="gB")
            nc.gpsimd.dma_gather(
                gB[:], tabB, idxB_t[:, jB0 * 8: jB1 * 8],
                num_idxs=JBr * 128, num_idxs_reg=JBr * 128, elem_size=ROW,
                single_packet=False)
            sB = spb.tile([128, JBr, 128], mybir.dt.bfloat16, tag="sB")
            nc.vector.tensor_tensor(
                sB[:],
                segB_t[:, jB0:jB1].unsqueeze(2).broadcast_to((128, JBr, 128)),
                iota_t[:].unsqueeze(1).broadcast_to((128, JBr, 128)),
                mybir.AluOpType.is_equal)

        for t in range(t0, t1):
            rows = min(128, meta.shard - t * 128)
            nga = int(meta.JA[t])
            ngb = int(meta.JB[t])
            ntot = nga + ngb
            if ntot == 0:
                continue
            pt = psum.tile([128, HID], mybir.dt.float32, tag="agg")
            k = 0
            for j in range(int(meta.offA[t]) - jA0, int(meta.offA[t]) - jA0 + nga):
                nc.tensor.matmul(pt[:], sA[:, j, :], gA[:, j, 0:96],
                                 start=(k == 0), stop=False)
                nc.tensor.matmul(pt[:], sA[:, j, :], gA[:, j, 96:192],
                                 start=False, stop=(k == ntot - 1))
                k += 1
            for j in range(int(meta.offB[t]) - jB0, int(meta.offB[t]) - jB0 + ngb):
                nc.tensor.matmul(pt[:], sB[:, j, :], gB[:, j, 0:96],
                                 start=(k == 0), stop=False)
                nc.tensor.matmul(pt[:], sB[:, j, :], gB[:, j, 96:192],
                                 start=False, stop=(k == ntot - 1))
                k += 1
            post_tile_fn(t, pt, rows)


def _load_agg_consts(nc, ctx, tc, meta, aps):
    cpool = ctx.enter_context(tc.tile_pool(name="aggc", bufs=1))
    consts = {}
    for nm, shp, dt in [
        ("idxA", [128, meta.sumJA * 8], mybir.dt.int16),
        ("idxB", [128, meta.sumJB * 8], mybir.dt.int16),
        ("segA", [128, meta.sumJA], mybir.dt.bfloat16),
        ("segB", [128, meta.sumJB], mybir.dt.bfloat16),
        ("iota", [128, 128], mybir.dt.bfloat16),
        ("dinv_sb", [128, meta.ntiles], mybir.dt.float32),
    ]:
        t = cpool.tile(shp, dt, tag=nm)
        nc.sync.dma_start(t[:], aps[nm][:])
        consts[nm] = t
    return consts


# ---------------- NEFF 1: layer-1 table build + aggregation ----------------
def build_neff1(meta, num_devices=NCORES, do_table=True, do_agg=True):
    n, shard, ntiles = meta.n, meta.shard, meta.ntiles
    nfull = math.ceil(n / 128)
    nc = bacc.Bacc("TRN2", target_bir_lowering=False, debug=False,
                   enable_asserts=False, num_devices=num_devices)
    aps = {}
    for nm, shp, dt in [
        ("xT", [F_IN, n], mybir.dt.float32),
        ("W1", [F_IN, HID], mybir.dt.float32),
        ("b1b", [128, HID], mybir.dt.float32),
        ("dinv_sb", [128, ntiles], mybir.dt.float32),
        ("iota", [128, 128], mybir.dt.bfloat16),
        ("idxA", [128, meta.sumJA * 8], mybir.dt.int16),
        ("idxB", [128, meta.sumJB * 8], mybir.dt.int16),
        ("segA", [128, meta.sumJA], mybir.dt.bfloat16),
        ("segB", [128, meta.sumJB], mybir.dt.bfloat16),
    ]:
        aps[nm] = nc.dram_tensor(nm, shp, dt, kind="ExternalInput").ap()
    x1_o = nc.dram_tensor("x1", [shard, HID], mybir.dt.float32,
                          kind="ExternalOutput").ap()
    x1p_o = nc.dram_tensor("x1p", [shard, ROW], mybir.dt.bfloat16,
                           kind="ExternalOutput").ap()
    table1 = nc.dram_tensor("table1", [n, ROW], mybir.dt.bfloat16,
                            kind="Internal").ap()

    with tile.TileContext(nc) as tc, ExitStack() as ctx:
        consts = _load_agg_consts(nc, ctx, tc, meta, aps)
        cpool = ctx.enter_context(tc.tile_pool(name="c1", bufs=1))
        w1_t = cpool.tile([F_IN, HID], mybir.dt.float32)
        nc.sync.dma_start(w1_t[:], aps["W1"][:])
        b1_t = cpool.tile([128, HID], mybir.dt.float32)
        nc.sync.dma_start(b1_t[:], aps["b1b"][:])

        xpool = ctx.enter_context(tc.tile_pool(name="xt", bufs=3))
        hps = ctx.enter_context(tc.tile_pool(name="hps", bufs=3, space="PSUM"))
        stp = ctx.enter_context(tc.tile_pool(name="stg", bufs=3))

        # ---- build table1 = ((dinv*x) @ W1) as split-bf16 rows ----
        for tt in range(nfull if do_table else 0):
            a = tt * 128
            cols = min(128, n - a)
            xt = xpool.tile([F_IN, 128], mybir.dt.float32, tag="xt")
            nc.sync.dma_start(xt[:, 0:cols], aps["xT"][:, a:a + cols])
            ph = hps.tile([128, HID], mybir.dt.float32, tag="h")
            nc.tensor.matmul(ph[0:cols, :], xt[:, 0:cols], w1_t[:],
                             start=True, stop=True)
            stg = stp.tile([128, ROW], mybir.dt.bfloat16, tag="stg")
            nc.scalar.activation(stg[0:cols, 0:96], ph[0:cols, :],
                                 mybir.ActivationFunctionType.Copy)
            nc.vector.tensor_tensor(stg[0:cols, 96:192], ph[0:cols, :],
                                    stg[0:cols, 0:96], mybir.AluOpType.subtract)
            nc.sync.dma_start(table1[a:a + cols, :], stg[0:cols, :])

        # ---- aggregate + finalize x1 / x1' ----
        opool = ctx.enter_context(tc.tile_pool(name="o1", bufs=3))

        def post(t, pt, rows):
            dv = consts["dinv_sb"][:, t:t + 1]
            u = opool.tile([128, HID], mybir.dt.float32, tag="u")
            nc.vector.scalar_tensor_tensor(
                u[0:rows, :], pt[0:rows, :], dv[0:rows, :], b1_t[0:rows, :],
                mybir.AluOpType.mult, mybir.AluOpType.add)
            x1t = opool.tile([128, HID], mybir.dt.float32, tag="x1t")
            nc.scalar.activation(x1t[0:rows, :], u[0:rows, :],
                                 mybir.ActivationFunctionType.Relu)
            nc.sync.dma_start(x1_o[t * 128: t * 128 + rows, :], x1t[0:rows, :])
            xp = opool.tile([128, HID], mybir.dt.float32, tag="xp")
            nc.vector.tensor_scalar_mul(xp[0:rows, :], x1t[0:rows, :], dv[0:rows, :])
            st2 = opool.tile([128, ROW], mybir.dt.bfloat16, tag="st2")
            nc.scalar.activation(st2[0:rows, 0:96], xp[0:rows, :],
                                 mybir.ActivationFunctionType.Copy)
            nc.vector.tensor_tensor(st2[0:rows, 96:192], xp[0:rows, :],
                                    st2[0:rows, 0:96], mybir.AluOpType.subtract)
            nc.sync.dma_start(x1p_o[t * 128: t * 128 + rows, :], st2[0:rows, :])

        if do_agg:
            _emit_aggregation(nc, ctx, tc, meta, table1, consts, post)
        else:
            # outputs must still be written
            z = opool.tile([128, ROW], mybir.dt.bfloat16, tag="st2")
            nc.vector.memset(z[:], 0.0)
            zf = opool.tile([128, HID], mybir.dt.float32, tag="x1t")
            nc.vector.memset(zf[:], 0.0)
            for t in range(ntiles):
                rows = min(128, shard - t * 128)
                nc.sync.dma_start(x1p_o[t * 128: t * 128 + rows, :], z[0:rows, :])
                nc.sync.dma_start(x1_o[t * 128: t * 128 + rows, :], zf[0:rows, :])
    nc.finalize()
    return nc


# ---------------- NEFF 2: layer-2 aggregation + W2 ----------------
def build_neff2(meta, num_devices=NCORES):
    n, shard, ntiles = meta.n, meta.shard, meta.ntiles
    nc = bacc.Bacc("TRN2", target_bir_lowering=False, debug=False,
                   enable_asserts=False, num_devices=num_devices)
    aps = {}
    for nm, shp, dt in [
        ("table2", [n, ROW], mybir.dt.bfloat16),
        ("W2", [HID, F_OUT], mybir.dt.float32),
        ("b2b", [128, F_OUT], mybir.dt.float32),
        ("ident", [128, 128], mybir.dt.float32),
        ("dinv_sb", [128, ntiles], mybir.dt.float32),
        ("iota", [128, 128], mybir.dt.bfloat16),
        ("idxA", [128, meta.sumJA * 8], mybir.dt.int16),
        ("idxB", [128, meta.sumJB * 8], mybir.dt.int16),
        ("segA", [128, meta.sumJA], mybir.dt.bfloat16),
        ("segB", [128, meta.sumJB], mybir.dt.bfloat16),
    ]:
        aps[nm] = nc.dram_tensor(nm, shp, dt, kind="ExternalInput").ap()
    out_o = nc.dram_tensor("out", [shard, F_OUT], mybir.dt.float32,
                           kind="ExternalOutput").ap()

    with tile.TileContext(nc) as tc, ExitStack() as ctx:
        consts = _load_agg_consts(nc, ctx, tc, meta, aps)
        cpool = ctx.enter_context(tc.tile_pool(name="c2", bufs=1))
        w2_t = cpool.tile([HID, F_OUT], mybir.dt.float32)
        nc.sync.dma_start(w2_t[:], aps["W2"][:])
        b2_t = cpool.tile([128, F_OUT], mybir.dt.float32)
        nc.sync.dma_start(b2_t[:], aps["b2b"][:])
        id_t = cpool.tile([128, 128], mybir.dt.float32)
        nc.sync.dma_start(id_t[:], aps["ident"][:])

        opool = ctx.enter_context(tc.tile_pool(name="o2", bufs=3))
        tps = ctx.enter_context(tc.tile_pool(name="tps", bufs=2, space="PSUM"))
        ops = ctx.enter_context(tc.tile_pool(name="ops", bufs=2, space="PSUM"))

        def post(t, pt, rows):
            dv = consts["dinv_sb"][:, t:t + 1]
            u = opool.tile([128, HID], mybir.dt.float32, tag="u")
            nc.vector.tensor_scalar_mul(u[0:rows, :], pt[0:rows, :], dv[0:rows, :])
            ptr = tps.tile([HID, 128], mybir.dt.float32, tag="tr")
            nc.tensor.transpose(ptr[0:HID, 0:rows], u[0:rows, 0:HID],
                                id_t[0:rows, 0:rows])
            uT = opool.tile([HID, 128], mybir.dt.float32, tag="uT")
            nc.vector.tensor_copy(uT[:, 0:rows], ptr[:, 0:rows])
            po = ops.tile([128, F_OUT], mybir.dt.float32, tag="po")
            nc.tensor.matmul(po[0:rows, :], uT[:, 0:rows], w2_t[:],
                             start=True, stop=True)
            ob = opool.tile([128, F_OUT], mybir.dt.float32, tag="ob")
            nc.vector.tensor_tensor(ob[0:rows, :], po[0:rows, :], b2_t[0:rows, :],
                                    mybir.AluOpType.add)
            nc.sync.dma_start(out_o[t * 128: t * 128 + rows, :], ob[0:rows, :])

        _emit_aggregation(nc, ctx, tc, meta, aps["table2"], consts, post)
    nc.finalize()
    return nc


# ---------------- orchestration ----------------
_CACHE = {}


def _run(meta, x, W1, b1, W2, b2, trace=False):
    n, shard, ncores = meta.n, meta.shard, meta.ncores
    key = ("neffs", n, ncores, meta.sumJA, meta.sumJB)
    if key not in _CACHE:
        _CACHE[key] = (build_neff1(meta, ncores), build_neff2(meta, ncores))
    nc1, nc2 = _CACHE[key]

    xT = np.ascontiguousarray((x * meta.dinv[:, None]).T.astype(np.float32))
    b1b = np.tile(np.asarray(b1, np.float32)[None, :], (128, 1))
    b2b = np.tile(np.asarray(b2, np.float32)[None, :], (128, 1))

    in1 = []
    for c in range(ncores):
        pc = meta.per_core[c]
        in1.append(dict(xT=xT, W1=np.asarray(W1, np.float32), b1b=b1b,
                        dinv_sb=pc["dinv_sb"], iota=meta.iota,
                        idxA=pc["idxA"], idxB=pc["idxB"],
                        segA=pc["segA"], segB=pc["segB"]))
    r1 = bass_utils.run_bass_kernel_spmd(nc1, in1, core_ids=list(range(ncores)),
                                         trace=trace)
    x1 = np.concatenate([r1.results[c]["x1"] for c in range(ncores)], axis=0)
    table2 = np.concatenate([r1.results[c]["x1p"] for c in range(ncores)], axis=0)

    in2 = []
    for c in range(ncores):
        pc = meta.per_core[c]
        in2.append(dict(table2=table2, W2=np.asarray(W2, np.float32), b2b=b2b,
                        ident=meta.ident, dinv_sb=pc["dinv_sb"], iota=meta.iota,
                        idxA=pc["idxA"], idxB=pc["idxB"],
                        segA=pc["segA"], segB=pc["segB"]))
    r2 = bass_utils.run_bass_kernel_spmd(nc2, in2, core_ids=list(range(ncores)),
                                         trace=trace)
    out = np.concatenate([r2.results[c]["out"] for c in range(ncores)], axis=0)
    return x1, out, r1, r2


def kernel(x, edge_index, W1, b1, W2, b2):
    x = np.asarray(x, np.float32)
    ei = np.asarray(edge_index)
    mkey = hashlib.sha1(np.ascontiguousarray(ei).tobytes()).hexdigest()
    if ("meta", mkey) not in _CACHE:
        _CACHE[("meta", mkey)] = build_meta(ei)
    meta = _CACHE[("meta", mkey)]
    x1, out, _, _ = _run(meta, x, W1, b1, W2, b2, trace=False)
    return x1, out
